# revision 4
# baseline (speedup 1.0000x reference)
"""int32-packed variant: each partition holds TWO batch rows as an fp16 pair
packed in one int32.  A clause needs only 32 partitions (2 groups of 16), so
one ap_gather instruction serves 4 index streams = 2 clauses x 2 chunk phases
-> 8 instructions of 4096 cols per step instead of 16 (Pool cost halved).

Partition layout: [0:32] clause A slots (rows q,q+32), [32:64] clause B,
[64:96] clause A duplicate (odd-phase chunks), [96:128] clause B duplicate.
Instruction j gathers chunk j for partitions 0-63 and chunk j+8 for 64-127.
Epilogue writes chunk j into dst[0:64] and chunk j+8 into dst[64:128]; two
SBUF->SBUF DMAs then mirror the halves so the next step's gathers see a full
table on every partition.  All reduces run in fp16 (2x DVE rate); host packs/
unpacks, so the device never converts dtypes.
"""

import numpy as np

GAMMA = 0.001
C, B, G, S, L = 16, 64, 2048, 8, 4
NCORES = 8
CPC = 2
P = 128
CHUNK_GI = 128
NCHUNK = 16               # logical chunks per step
NINST = 8                 # gather instructions per step (2 chunks each)
CH_COLS = CHUNK_GI * S * L  # 4096 int32 cols per chunk
IDXC = CH_COLS // 16        # 256 idx cols per partition per instruction

_nc_cache = {}


def _build(steps: int):
    import concourse.bacc as bacc
    import concourse.mybir as mybir
    import concourse.tile as tile

    f16 = mybir.dt.float16
    i32 = mybir.dt.int32
    i16 = mybir.dt.int16
    ALU = mybir.AluOpType

    nc = bacc.Bacc("TRN2", target_bir_lowering=False)
    xin = nc.dram_tensor("xin", [P, G], i32, kind="ExternalInput")
    idxin = nc.dram_tensor("idxin", [P, NINST * IDXC], i16, kind="ExternalInput")
    outd = nc.dram_tensor("outd", [P, G], i32, kind="ExternalOutput")

    with tile.TileContext(nc) as tc:
        with (
            tc.tile_pool(name="state", bufs=1) as st,
            tc.tile_pool(name="work", bufs=2) as wp,
            tc.tile_pool(name="small", bufs=2) as sp,
        ):
            Rbuf = [st.tile([P, G], i32, tag=f"R{i}", name=f"R{i}") for i in range(2)]
            IDX = st.tile([P, NINST * IDXC], i16, tag="IDX")
            nc.sync.dma_start(out=Rbuf[0][:], in_=xin.ap())
            nc.sync.dma_start(out=IDX[:], in_=idxin.ap())

            for t in range(steps):
                src = Rbuf[t % 2]
                dst = Rbuf[(t + 1) % 2]
                srch = src[:].bitcast(f16)   # [P, 2*G]
                dsth = dst[:].bitcast(f16)
                for j in range(NINST):
                    g = wp.tile([P, CH_COLS], i32, tag="g")
                    nc.gpsimd.ap_gather(
                        g[:], src[:], IDX[:, j * IDXC : (j + 1) * IDXC],
                        channels=P, num_elems=G, d=1, num_idxs=CH_COLS,
                    )
                    gh = g[:].bitcast(f16)   # [P, 8192] (l,s,gi) int32-col l-major
                    m1 = sp.tile([P, 4096], f16, tag="m1")
                    nc.vector.tensor_tensor(out=m1[:], in0=gh[:, 0:4096], in1=gh[:, 4096:8192], op=ALU.min)
                    m2 = sp.tile([P, 2048], f16, tag="m2")
                    nc.vector.tensor_tensor(out=m2[:], in0=m1[:, 0:2048], in1=m1[:, 2048:4096], op=ALU.min)
                    t1 = sp.tile([P, 1024], f16, tag="t1")
                    nc.vector.tensor_tensor(out=t1[:], in0=m2[:, 0:1024], in1=m2[:, 1024:2048], op=ALU.max)
                    t2 = sp.tile([P, 512], f16, tag="t2")
                    nc.vector.tensor_tensor(out=t2[:], in0=t1[:, 0:512], in1=t1[:, 512:1024], op=ALU.max)
                    t3 = sp.tile([P, 256], f16, tag="t3")
                    nc.vector.tensor_tensor(out=t3[:], in0=t2[:, 0:256], in1=t2[:, 256:512], op=ALU.max)
                    lo = slice(j * 256, (j + 1) * 256)
                    hi = slice((j + 8) * 256, (j + 9) * 256)
                    nc.vector.tensor_tensor(out=dsth[0:64, lo], in0=srch[0:64, lo], in1=t3[0:64, :], op=ALU.max)
                    nc.vector.tensor_tensor(out=dsth[64:128, hi], in0=srch[64:128, hi], in1=t3[64:128, :], op=ALU.max)
                if t != steps - 1:
                    # mirror halves: full table everywhere for next step
                    nc.sync.dma_start(out=dst[64:128, 0:1024], in_=dst[0:64, 0:1024])
                    nc.sync.dma_start(out=dst[0:64, 1024:2048], in_=dst[64:128, 1024:2048])

            nc.sync.dma_start(out=outd.ap(), in_=Rbuf[steps % 2][:])

    nc.compile()
    return nc


def _wrap(cols):
    return cols.astype(np.int16).reshape(IDXC, 16).T


def _make_inputs(x: np.ndarray, I: np.ndarray):
    xh = x.astype(np.float16)
    pack = np.empty((32, G, 2), dtype=np.float16)
    pack[:, :, 0] = xh[0:32]
    pack[:, :, 1] = xh[32:64]
    packv = pack.reshape(32, 2 * G).view(np.int32)      # (32, G) int32
    xin = np.concatenate([packv, packv, packv, packv], axis=0)  # A,B,A,B (x same)

    in_maps = []
    for core in range(NCORES):
        idx_full = np.empty((P, NINST * IDXC), dtype=np.int16)
        cls = [I[core * CPC], I[core * CPC + 1]]
        for j in range(NINST):
            colslice = slice(j * IDXC, (j + 1) * IDXC)
            for phase in range(2):   # 0: chunk j -> partitions 0-63; 1: chunk j+8 -> 64-127
                k = j + 8 * phase
                for cl in range(2):
                    sub = cls[cl][k * CHUNK_GI : (k + 1) * CHUNK_GI]   # (gi,S,L)
                    w = _wrap(sub.transpose(2, 1, 0).reshape(-1))      # l-major
                    base = 64 * phase + 32 * cl
                    idx_full[base : base + 16, colslice] = w
                    idx_full[base + 16 : base + 32, colslice] = w
        in_maps.append({"xin": xin, "idxin": idx_full})
    return in_maps


def kernel(x: np.ndarray, I: np.ndarray, infer_step) -> np.ndarray:
    from concourse import bass_utils

    steps = int(infer_step)
    x = np.asarray(x, dtype=np.float32)
    I = np.asarray(I, dtype=np.int32)
    if steps not in _nc_cache:
        _nc_cache[steps] = _build(steps)
    nc = _nc_cache[steps]

    in_maps = _make_inputs(x, I)
    res = bass_utils.run_bass_kernel_spmd(nc, in_maps, list(range(NCORES)))
    out = np.empty((C, B, G), dtype=np.float32)
    v = np.arange(G)
    half = (v // CHUNK_GI >= 8).astype(np.int64)        # chunk phase of column v
    # final table: chunks 0-7 valid on partitions 0-63, chunks 8-15 on 64-127
    for core in range(NCORES):
        o = np.asarray(res.results[core]["outd"]).view(np.int32).reshape(P, G)
        fh = o.view(np.float16).reshape(P, G, 2)        # [p, v, lane]
        for cl in range(2):
            rows = np.empty((B, G), dtype=np.float32)
            for lane in range(2):
                q = np.arange(32)
                p_idx = (64 * half)[None, :] + 32 * cl + q[:, None]   # (32, G)
                rows[lane * 32 : (lane + 1) * 32] = fh[p_idx, v[None, :], lane]
            out[core * CPC + cl] = rows
    return out




# revision 9
# speedup vs baseline: 1.0327x; 1.0327x over previous
"""int32-packed variant: each partition holds TWO batch rows as an fp16 pair
packed in one int32.  A clause needs only 32 partitions (2 groups of 16), so
one ap_gather instruction serves 4 index streams = 2 clauses x 2 chunk phases
-> 8 instructions of 4096 cols per step instead of 16 (Pool cost halved).

Partition layout: [0:32] clause A slots (rows q,q+32), [32:64] clause B,
[64:96] clause A duplicate (odd-phase chunks), [96:128] clause B duplicate.
Instruction j gathers chunk j for partitions 0-63 and chunk j+8 for 64-127.
Epilogue writes chunk j into dst[0:64] and chunk j+8 into dst[64:128]; two
SBUF->SBUF DMAs then mirror the halves so the next step's gathers see a full
table on every partition.  All reduces run in fp16 (2x DVE rate); host packs/
unpacks, so the device never converts dtypes.
"""

import numpy as np

GAMMA = 0.001
C, B, G, S, L = 16, 64, 2048, 8, 4
NCORES = 8
CPC = 2
P = 128
CHUNK_GI = 128
NCHUNK = 16               # logical chunks per step
NINST = 8                 # gather instructions per step (2 chunks each)
CH_COLS = CHUNK_GI * S * L  # 4096 int32 cols per chunk
IDXC = CH_COLS // 16        # 256 idx cols per partition per instruction

_nc_cache = {}


def _build(steps: int):
    import concourse.bacc as bacc
    import concourse.mybir as mybir
    import concourse.tile as tile

    f16 = mybir.dt.float16
    i32 = mybir.dt.int32
    i16 = mybir.dt.int16
    ALU = mybir.AluOpType

    nc = bacc.Bacc("TRN2", target_bir_lowering=False)
    xin = nc.dram_tensor("xin", [P, G], i32, kind="ExternalInput")
    idxin = nc.dram_tensor("idxin", [P, NINST * IDXC], i16, kind="ExternalInput")
    outd = nc.dram_tensor("outd", [P, G // 2], i32, kind="ExternalOutput")

    with tile.TileContext(nc) as tc:
        with (
            tc.tile_pool(name="state", bufs=1) as st,
            tc.tile_pool(name="work", bufs=2) as wp,
            tc.tile_pool(name="small", bufs=2) as sp,
        ):
            Rbuf = [st.tile([P, G], i32, tag=f"R{i}", name=f"R{i}") for i in range(2)]
            IDX = st.tile([P, NINST * IDXC], i16, tag="IDX")
            nc.sync.dma_start(out=Rbuf[0][:], in_=xin.ap())
            nc.sync.dma_start(out=IDX[:], in_=idxin.ap())

            def emit_inst(src, dst, srch, dsth, icol, j, g0, g1):
                ng = g1 - g0                      # gi count (<=128)
                nc32 = ng * 32                    # int32 gather cols
                g = wp.tile([P, nc32], i32, tag="g", name="g")
                nc.gpsimd.ap_gather(
                    g[:], src[:], IDX[:, icol : icol + nc32 // 16],
                    channels=P, num_elems=G, d=1, num_idxs=nc32,
                )
                cur = g[:].bitcast(f16)           # [P, ng*64] l-major
                w = ng * 32
                for lvl, op in enumerate((ALU.min, ALU.min, ALU.max, ALU.max, ALU.max)):
                    nxt = sp.tile([P, w], f16, tag=f"r{lvl}", name=f"r{lvl}")
                    nc.vector.tensor_tensor(out=nxt[:], in0=cur[:, 0:w], in1=cur[:, w : 2 * w], op=op)
                    cur, w = nxt[:], w // 2
                t3 = cur                           # [P, ng*2] fp16
                lo = slice(j * 256 + g0 * 2, j * 256 + g1 * 2)
                hi = slice((j + 8) * 256 + g0 * 2, (j + 8) * 256 + g1 * 2)
                nc.vector.tensor_tensor(out=dsth[0:64, lo], in0=srch[0:64, lo], in1=t3[0:64, :], op=ALU.max)
                nc.vector.tensor_tensor(out=dsth[64:128, hi], in0=srch[64:128, hi], in1=t3[64:128, :], op=ALU.max)

            for t in range(steps):
                src = Rbuf[t % 2]
                dst = Rbuf[(t + 1) % 2]
                srch = src[:].bitcast(f16)   # [P, 2*G]
                dsth = dst[:].bitcast(f16)
                icol = 0
                for j in range(NINST - 1):
                    emit_inst(src, dst, srch, dsth, icol, j, 0, 128)
                    icol += IDXC
                    if j == 3 and t != steps - 1:
                        # first-quarter mirrors overlap remaining compute
                        nc.sync.dma_start(out=dst[64:128, 0:512], in_=dst[0:64, 0:512])
                        nc.sync.dma_start(out=dst[0:64, 1024:1536], in_=dst[64:128, 1024:1536])
                emit_inst(src, dst, srch, dsth, icol, NINST - 1, 0, 64)
                icol += IDXC // 2
                emit_inst(src, dst, srch, dsth, icol, NINST - 1, 64, 128)
                if t != steps - 1:
                    nc.sync.dma_start(out=dst[64:128, 512:1024], in_=dst[0:64, 512:1024])
                    nc.sync.dma_start(out=dst[0:64, 1536:2048], in_=dst[64:128, 1536:2048])

            fin = Rbuf[steps % 2]
            nc.sync.dma_start(out=outd.ap()[0:64, :], in_=fin[0:64, 0:1024])
            nc.sync.dma_start(out=outd.ap()[64:128, :], in_=fin[64:128, 1024:2048])

    nc.compile()
    return nc


def _wrap(cols):
    return cols.astype(np.int16).reshape(-1, 16).T


def _make_inputs(x: np.ndarray, I: np.ndarray):
    xh = x.astype(np.float16)
    pack = np.empty((32, G, 2), dtype=np.float16)
    pack[:, :, 0] = xh[0:32]
    pack[:, :, 1] = xh[32:64]
    packv = pack.reshape(32, 2 * G).view(np.int32)      # (32, G) int32
    xin = np.concatenate([packv, packv, packv, packv], axis=0)  # A,B,A,B (x same)

    # instruction list: 7 full 128-gi instructions, then the last chunk pair
    # split into two 64-gi halves (shorter end-of-step DVE tail)
    insts = [(j, 0, 128) for j in range(NINST - 1)]
    insts += [(NINST - 1, 0, 64), (NINST - 1, 64, 128)]

    in_maps = []
    for core in range(NCORES):
        idx_full = np.empty((P, NINST * IDXC), dtype=np.int16)
        cls = [I[core * CPC], I[core * CPC + 1]]
        icol = 0
        for j, g0, g1 in insts:
            ncol = (g1 - g0) * 32 // 16
            colslice = slice(icol, icol + ncol)
            icol += ncol
            for phase in range(2):   # 0: chunk j -> partitions 0-63; 1: chunk j+8 -> 64-127
                k = j + 8 * phase
                for cl in range(2):
                    sub = cls[cl][k * CHUNK_GI + g0 : k * CHUNK_GI + g1]  # (gi,S,L)
                    w = _wrap(sub.transpose(2, 1, 0).reshape(-1))         # l-major
                    base = 64 * phase + 32 * cl
                    idx_full[base : base + 16, colslice] = w
                    idx_full[base + 16 : base + 32, colslice] = w
        in_maps.append({"xin": xin, "idxin": idx_full})
    return in_maps


def kernel(x: np.ndarray, I: np.ndarray, infer_step) -> np.ndarray:
    from concourse import bass_utils

    steps = int(infer_step)
    x = np.asarray(x, dtype=np.float32)
    I = np.asarray(I, dtype=np.int32)
    if steps not in _nc_cache:
        _nc_cache[steps] = _build(steps)
    nc = _nc_cache[steps]

    in_maps = _make_inputs(x, I)
    res = bass_utils.run_bass_kernel_spmd(nc, in_maps, list(range(NCORES)))
    out = np.empty((C, B, G), dtype=np.float32)
    # outd [128, 1024] int32: rows 0-63 = cols 0:1024 (chunks 0-7),
    # rows 64-127 = cols 1024:2048 (chunks 8-15)
    for core in range(NCORES):
        o = np.asarray(res.results[core]["outd"]).view(np.int32).reshape(P, G // 2)
        fh = o.view(np.float16).reshape(P, G // 2, 2)   # [p, v_local, lane]
        for cl in range(2):
            rows = np.empty((B, G), dtype=np.float32)
            for lane in range(2):
                r0 = lane * 32
                rows[r0 : r0 + 32, 0:1024] = fh[32 * cl : 32 * cl + 32, :, lane]
                rows[r0 : r0 + 32, 1024:2048] = fh[64 + 32 * cl : 64 + 32 * cl + 32, :, lane]
            out[core * CPC + cl] = rows
    return out


